# revision 13
# baseline (speedup 1.0000x reference)
"""Trainium2 Bass kernel for BinarySplitDecoder (binary-tree leaf probabilities).

Contract: kernel(x) takes the FULL input x [65536, 1023] fp32 and returns the
FULL output [65536, 1024] fp32 (leaf probabilities of a depth-10 binary split
tree, level-major node ordering).

Sharding: pure data parallel — batch dim split evenly across 8 NeuronCores.

Strategy (fp16, block layout, host-side repack for contiguous DMA):
  - Host casts x to fp16 and permutes columns (within each tree level, a
    bit-reversal involution); the device returns fp16 leaves in bit-reversed
    ("block") order, which the host un-permutes + casts back to fp32. The
    2e-2 relative-error gate makes fp16 safe (measured ~1.5e-3).
  - Block layout: each tree step writes left children into a packed lower
    half and right children into a packed upper half. Packed 2-byte step-1
    operands keep every tensor_tensor in the DVE 2x_1p perf mode.
    right = cur - left replaces cur * (1 - a).
  - Row p*64+u of a core's batch lives on partition p, unit u. The host
    packs every tensor so each DMA is per-partition contiguous on BOTH the
    DRAM and SBUF side. Descriptor size is the whole game (HW-measured):
    >=7 KB descriptors sustain ~430 GB/s; 4 KB descriptors drain at ~100
    GB/s (~600 ns serialized completion each); the original [B, 1023] row
    layout forced 0.5-1 KB descriptors and capped loads at ~310 GB/s.
  - Input splits: xs (levels 0-5, 63 cols), x68 (levels 6-8, 448 cols),
    x9 (level 9, 512 cols). Two-stage walk: stage A runs levels 0-5 for
    all 64 units in 12 DVE ops; stage B runs levels 6-9 per chunk of g
    units. 52 DVE ops total (~76 us busy; each op's fixed cost is ~160 ns).
  - Pipelining: x68 pool bufs=3 (chunk data prefetched 2 ahead, compute
    never waits), x9 pool bufs=2 (level-9 alphas WAR-paced one chunk ahead
    -- also spaces the load stream so it does not hog the fabric), outL and
    outR pools bufs=3 (a store can lag a whole chunk without blocking
    compute via WAR). Loads + the shallow piece issue on the ACT HWDGE
    ring, stores on the SP ring: each ring is FIFO, so store waits never
    block loads. Unpaced back-to-back loads must be avoided: a schedule
    that sustained ~490 GB/s for >20 us tripped a 0.96 -> 0.8 GHz downclock
    of the whole core (every DVE op exactly 1.2x slower, persisting in
    zero-DMA windows).
  - Level-8 output goes straight into the out-right tile; the level-9
    subtract runs in place on it. The left half stores while the subtract
    computes. Output DRAM layout is per-chunk [left block | right block],
    both dense, unscrambled on the host.
  - Stage-A ping-pong tiles borrow the stage-B pool slots (tags B0/B1);
    stage A is done before stage B's first WAR on them. Total SBUF
    ~202.6 KB of the ~208 KB usable.
"""

import numpy as np

import concourse.bacc as bacc
import concourse.bass as bass
import concourse.mybir as mybir
from concourse.tile import TileContext
from concourse.bass_utils import run_bass_kernel_spmd

TREE_DEPTH = 10
N_NODES = (1 << TREE_DEPTH) - 1  # 1023
N_LEAVES = 1 << TREE_DEPTH  # 1024
N_CORES = 8
P = 128  # SBUF partitions
U = 64  # row-units per core: 8192 rows / 128 partitions
SPLIT_D = 6  # levels 0..5 in stage A, 6..9 in stage B
NS = (1 << SPLIT_D) - 1  # 63 shallow alpha cols per unit
N68 = 448  # levels 6-8 alpha cols per unit
N9 = 512  # level-9 alpha cols per unit
H = N_LEAVES // 2  # 512
CHUNKS = (8, 16, 16, 16, 8)  # units per stage-B chunk; sums to U
OFFS = tuple(int(v) for v in np.concatenate([[0], np.cumsum(CHUNKS)[:-1]]))


def _bitrev(n: int, bits: int) -> int:
    r = 0
    for _ in range(bits):
        r = (r << 1) | (n & 1)
        n >>= 1
    return r


def _col_perm() -> np.ndarray:
    """xp[:, base+p] = x[:, base+rev_s(p)]: per-level bit-reversal so the
    block-layout walk consumes alphas from contiguous slices."""
    perm = np.arange(N_NODES)
    for s in range(TREE_DEPTH):
        base = (1 << s) - 1
        for p in range(1 << s):
            perm[base + p] = base + _bitrev(p, s)
    return perm


COL_PERM = _col_perm()
# block position j holds standard leaf rev(j); rev is an involution
OUT_PERM = np.array([_bitrev(m, TREE_DEPTH) for m in range(N_LEAVES)])


def build_nc() -> bass.Bass:
    """Per-core Bass program:
      DRAM "xs"  [P, U, 63]  fp16 — levels 0-5 alphas (col-permuted)
      DRAM "x68" [P, U, 448] fp16 — levels 6-8 alphas
      DRAM "x9"  [P, U, 512] fp16 — level-9 alphas
      DRAM "y"   [P, U*1024] fp16 — per-chunk [left block | right block]
    """
    f16 = mybir.dt.float16

    nc = bacc.Bacc("TRN2", target_bir_lowering=False, debug=False)
    xs = nc.declare_dram_parameter("xs", [P, U, NS], f16, isOutput=False)
    x68 = nc.declare_dram_parameter("x68", [P, U, N68], f16, isOutput=False)
    x9 = nc.declare_dram_parameter("x9", [P, U, N9], f16, isOutput=False)
    y = nc.declare_dram_parameter("y", [P, U * N_LEAVES], f16, isOutput=True)

    with TileContext(nc) as tc:
        with (
            tc.tile_pool(name="xsp", bufs=1) as sp,
            tc.tile_pool(name="x68p", bufs=3) as x68p,
            tc.tile_pool(name="x9p", bufs=2) as x9p,
            tc.tile_pool(name="c5p", bufs=1) as c5p,
            tc.tile_pool(name="curB", bufs=1) as cbp,
            # level-6 outputs: bufs=2 so GpSimd can write chunk c+1's
            # level 6 while DVE still reads chunk c's
            tc.tile_pool(name="c6p", bufs=2) as c6p,
            tc.tile_pool(name="outL", bufs=3) as olp,
            tc.tile_pool(name="outR", bufs=3) as orp,
        ):
            st = sp.tile([P, U, NS], f16, name="st")
            nc.scalar.dma_start(out=st, in_=xs[:, :, :])

            t68, t9 = {}, {}

            def load68(c):
                if c >= len(CHUNKS):
                    return
                g, off = CHUNKS[c], OFFS[c]
                t68[c] = x68p.tile([P, g, N68], f16, tag="x68", name=f"x68_{c}")
                nc.scalar.dma_start(out=t68[c], in_=x68[:, off : off + g, :])

            def load9(c):
                if c >= len(CHUNKS):
                    return
                g, off = CHUNKS[c], OFFS[c]
                t9[c] = x9p.tile([P, g, N9], f16, tag="x9", name=f"x9_{c}")
                nc.scalar.dma_start(out=t9[c], in_=x9[:, off : off + g, :])

            load68(0)
            load9(0)
            load68(1)
            load9(1)

            # stage A: levels 0..5 for all 64 units at once
            c5 = c5p.tile([P, U, 1 << SPLIT_D], f16, name="c5")
            cur = None
            for d in range(SPLIT_D):
                L = 1 << d
                if d == SPLIT_D - 1:
                    nxt = c5
                else:
                    # stage-A ping-pong borrows stage-B slots (even levels
                    # the c6 pool, odd levels tag B1); stage A is done
                    # before stage B's first WAR on them
                    if d % 2 == 0:
                        nxt = c6p.tile([P, U, 2 * L], f16, tag="c6")
                    else:
                        nxt = cbp.tile([P, U, 2 * L], f16, tag="B1")
                left = nxt[:, :, 0:L]
                right = nxt[:, :, L : 2 * L]
                a = st[:, :, L - 1 : 2 * L - 1]
                if d == 0:
                    nc.vector.tensor_copy(out=left, in_=a)
                    nc.vector.tensor_scalar(
                        out=right,
                        in0=a,
                        scalar1=-1.0,
                        scalar2=1.0,
                        op0=mybir.AluOpType.mult,
                        op1=mybir.AluOpType.add,
                    )
                else:
                    nc.vector.tensor_mul(out=left, in0=cur, in1=a)
                    nc.vector.tensor_tensor(
                        out=right, in0=cur, in1=left, op=mybir.AluOpType.subtract
                    )
                cur = nxt

            # stage B: levels 6..9 per chunk of g units. Level 6 of chunk
            # c+1 runs on GpSimd (otherwise idle; ~3.7x slower per element
            # but fully hidden under DVE's levels 7-9 of chunk c), so DVE
            # only computes levels 7-9 for chunks >= 1.
            c6t = {}

            def gpsimd_level6(c):
                if c >= len(CHUNKS) or c in c6t:
                    return
                g1, off1 = CHUNKS[c], OFFS[c]
                t = c6p.tile([P, g1, 128], f16, tag="c6", name=f"c6_{c}")
                a6 = t68[c][:, :, 0:64]
                cur1 = c5[:, off1 : off1 + g1, :]
                nc.gpsimd.tensor_mul(out=t[:, :, 0:64], in0=cur1, in1=a6)
                nc.gpsimd.tensor_tensor(
                    out=t[:, :, 64:128],
                    in0=cur1,
                    in1=t[:, :, 0:64],
                    op=mybir.AluOpType.subtract,
                )
                c6t[c] = t

            for c, g in enumerate(CHUNKS):
                off = OFFS[c]
                load68(c + 2)
                load9(c + 2)
                gpsimd_level6(c + 1)
                xt = t68.pop(c)
                x9t = t9.pop(c)
                outL = olp.tile([P, g, H], f16, tag="yl")
                outR = orp.tile([P, g, H], f16, tag="yr")
                cur = c5[:, off : off + g, :]
                for d in range(SPLIT_D, TREE_DEPTH):
                    L = 1 << d
                    if d == SPLIT_D:
                        if c == 0:
                            t = c6p.tile([P, g, 128], f16, tag="c6", name="c6_0")
                            a = xt[:, :, 0:64]
                            nc.vector.tensor_mul(out=t[:, :, 0:64], in0=cur, in1=a)
                            nc.vector.tensor_tensor(
                                out=t[:, :, 64:128],
                                in0=cur,
                                in1=t[:, :, 0:64],
                                op=mybir.AluOpType.subtract,
                            )
                        else:
                            t = c6t.pop(c)
                        cur = t
                    elif d == TREE_DEPTH - 1:
                        a = x9t[:, :, :]
                        # left half of the leaves is final after this mul:
                        # drain it while the subtract computes the right half
                        nc.vector.tensor_mul(out=outL, in0=cur, in1=a)
                        base = off * N_LEAVES
                        nc.sync.dma_start(
                            out=y[:, base : base + g * H].rearrange(
                                "p (u m) -> p u m", u=g, m=H
                            ),
                            in_=outL,
                        )
                        # in-place: per-element read precedes write on DVE
                        nc.vector.tensor_tensor(
                            out=outR, in0=cur, in1=outL, op=mybir.AluOpType.subtract
                        )
                        nc.sync.dma_start(
                            out=y[:, base + g * H : base + 2 * g * H].rearrange(
                                "p (u m) -> p u m", u=g, m=H
                            ),
                            in_=outR,
                        )
                    elif d == TREE_DEPTH - 2:
                        a = xt[:, :, L - 64 : 2 * L - 64]
                        # level-8 output goes straight into the out-right tile
                        left = outR[:, :, 0:L]
                        right = outR[:, :, L : 2 * L]
                        nc.vector.tensor_mul(out=left, in0=cur, in1=a)
                        nc.vector.tensor_tensor(
                            out=right, in0=cur, in1=left, op=mybir.AluOpType.subtract
                        )
                        cur = outR
                    else:
                        # d == 7
                        a = xt[:, :, L - 64 : 2 * L - 64]
                        nxt = cbp.tile([P, g, 2 * L], f16, tag="B1")
                        left = nxt[:, :, 0:L]
                        right = nxt[:, :, L : 2 * L]
                        nc.vector.tensor_mul(out=left, in0=cur, in1=a)
                        nc.vector.tensor_tensor(
                            out=right, in0=cur, in1=left, op=mybir.AluOpType.subtract
                        )
                        cur = nxt

    nc.compile()
    return nc


def _run(x: np.ndarray, **spmd_kwargs):
    """Shard x, run the Bass kernel on all 8 cores, return (y, BassKernelResults)."""
    x = np.asarray(x, dtype=np.float32)
    B = x.shape[0]
    assert B % N_CORES == 0 and x.shape[1] == N_NODES
    rpc = B // N_CORES
    assert rpc == P * U

    xh = np.ascontiguousarray(x[:, COL_PERM].astype(np.float16))

    nc = build_nc()
    in_maps = []
    for i in range(N_CORES):
        x3 = xh[i * rpc : (i + 1) * rpc].reshape(P, U, N_NODES)
        in_maps.append(
            {
                "xs": np.ascontiguousarray(x3[:, :, :NS]),
                "x68": np.ascontiguousarray(x3[:, :, NS : NS + N68]),
                "x9": np.ascontiguousarray(x3[:, :, NS + N68 :]),
            }
        )
    res = run_bass_kernel_spmd(nc, in_maps, list(range(N_CORES)), **spmd_kwargs)

    outs = []
    for r in res.results:
        yd = r["y"].reshape(P, U * N_LEAVES)
        yb = np.empty((P, U, N_LEAVES), dtype=np.float16)
        for c, g in enumerate(CHUNKS):
            u0 = OFFS[c]
            seg = yd[:, u0 * N_LEAVES : (u0 + g) * N_LEAVES].reshape(P, 2, g, H)
            yb[:, u0 : u0 + g, 0:H] = seg[:, 0]
            yb[:, u0 : u0 + g, H:] = seg[:, 1]
        outs.append(yb.reshape(rpc, N_LEAVES))
    out = np.concatenate(outs, axis=0)
    out = out[:, OUT_PERM].astype(np.float32)
    return out, res


def kernel(x: np.ndarray) -> np.ndarray:
    return _run(x)[0]


# revision 14
# speedup vs baseline: 1.0471x; 1.0471x over previous
"""Trainium2 Bass kernel for BinarySplitDecoder (binary-tree leaf probabilities).

Contract: kernel(x) takes the FULL input x [65536, 1023] fp32 and returns the
FULL output [65536, 1024] fp32 (leaf probabilities of a depth-10 binary split
tree, level-major node ordering).

Sharding: pure data parallel — batch dim split evenly across 8 NeuronCores.

Strategy (fp16, block layout, host-side repack for contiguous DMA):
  - Host casts x to fp16 and permutes columns (within each tree level, a
    bit-reversal involution); the device returns fp16 leaves in bit-reversed
    ("block") order, which the host un-permutes + casts back to fp32. The
    2e-2 relative-error gate makes fp16 safe (measured ~1.5e-3).
  - Block layout: each tree step writes left children into a packed lower
    half and right children into a packed upper half. Packed 2-byte step-1
    operands keep every tensor_tensor in the DVE 2x_1p perf mode.
    right = cur - left replaces cur * (1 - a).
  - Row p*64+u of a core's batch lives on partition p, unit u. The host
    packs every tensor so each DMA is per-partition contiguous on BOTH the
    DRAM and SBUF side. Descriptor size is the whole game (HW-measured):
    >=7 KB descriptors sustain ~430 GB/s; 4 KB descriptors drain at ~100
    GB/s (~600 ns serialized completion each); the original [B, 1023] row
    layout forced 0.5-1 KB descriptors and capped loads at ~310 GB/s.
  - Input splits: xs (levels 0-5, 63 cols), x68 (levels 6-8, 448 cols),
    x9 (level 9, 512 cols). Two-stage walk: stage A runs levels 0-5 for
    all 64 units in 12 DVE ops; stage B runs levels 6-9 per chunk of g
    units. 52 DVE ops total (~76 us busy; each op's fixed cost is ~160 ns).
  - Pipelining: x68 pool bufs=3 (chunk data prefetched 2 ahead, compute
    never waits), x9 pool bufs=2 (level-9 alphas WAR-paced one chunk ahead
    -- also spaces the load stream so it does not hog the fabric), outL and
    outR pools bufs=3 (a store can lag a whole chunk without blocking
    compute via WAR). Loads + the shallow piece issue on the ACT HWDGE
    ring, stores on the SP ring: each ring is FIFO, so store waits never
    block loads. Unpaced back-to-back loads must be avoided: a schedule
    that sustained ~490 GB/s for >20 us tripped a 0.96 -> 0.8 GHz downclock
    of the whole core (every DVE op exactly 1.2x slower, persisting in
    zero-DMA windows).
  - Level-8 output goes straight into the out-right tile; the level-9
    subtract runs in place on it. The left half stores while the subtract
    computes. Output DRAM layout is per-chunk [left block | right block],
    both dense, unscrambled on the host.
  - Stage-A ping-pong tiles borrow the stage-B pool slots (tags B0/B1);
    stage A is done before stage B's first WAR on them. Total SBUF
    ~202.6 KB of the ~208 KB usable.
"""

import numpy as np

import concourse.bacc as bacc
import concourse.bass as bass
import concourse.mybir as mybir
from concourse.tile import TileContext
from concourse.bass_utils import run_bass_kernel_spmd

TREE_DEPTH = 10
N_NODES = (1 << TREE_DEPTH) - 1  # 1023
N_LEAVES = 1 << TREE_DEPTH  # 1024
N_CORES = 8
P = 128  # SBUF partitions
U = 64  # row-units per core: 8192 rows / 128 partitions
SPLIT_D = 6  # levels 0..5 in stage A, 6..9 in stage B
NS = (1 << SPLIT_D) - 1  # 63 shallow alpha cols per unit
N68 = 448  # levels 6-8 alpha cols per unit
N9 = 512  # level-9 alpha cols per unit
H = N_LEAVES // 2  # 512
CHUNKS = (8, 16, 16, 16, 8)  # units per stage-B chunk; sums to U
OFFS = tuple(int(v) for v in np.concatenate([[0], np.cumsum(CHUNKS)[:-1]]))


def _bitrev(n: int, bits: int) -> int:
    r = 0
    for _ in range(bits):
        r = (r << 1) | (n & 1)
        n >>= 1
    return r


def _col_perm() -> np.ndarray:
    """xp[:, base+p] = x[:, base+rev_s(p)]: per-level bit-reversal so the
    block-layout walk consumes alphas from contiguous slices."""
    perm = np.arange(N_NODES)
    for s in range(TREE_DEPTH):
        base = (1 << s) - 1
        for p in range(1 << s):
            perm[base + p] = base + _bitrev(p, s)
    return perm


COL_PERM = _col_perm()
# block position j holds standard leaf rev(j); rev is an involution
OUT_PERM = np.array([_bitrev(m, TREE_DEPTH) for m in range(N_LEAVES)])


def build_nc() -> bass.Bass:
    """Per-core Bass program:
      DRAM "xs"  [P, U, 63]  fp16 — levels 0-5 alphas (col-permuted)
      DRAM "x68" [P, U, 448] fp16 — levels 6-8 alphas
      DRAM "x9"  [P, U, 512] fp16 — level-9 alphas
      DRAM "y"   [P, U*1024] fp16 — per-chunk [left block | right block]
    """
    f16 = mybir.dt.float16

    nc = bacc.Bacc("TRN2", target_bir_lowering=False, debug=False)
    xs = nc.declare_dram_parameter("xs", [P, U, NS], f16, isOutput=False)
    x68 = nc.declare_dram_parameter("x68", [P, U, N68], f16, isOutput=False)
    x9 = nc.declare_dram_parameter("x9", [P, U, N9], f16, isOutput=False)
    y = nc.declare_dram_parameter("y", [P, U * N_LEAVES], f16, isOutput=True)

    with TileContext(nc) as tc:
        with (
            tc.tile_pool(name="xsp", bufs=1) as sp,
            tc.tile_pool(name="x68p", bufs=3) as x68p,
            tc.tile_pool(name="x9p", bufs=2) as x9p,
            tc.tile_pool(name="c5p", bufs=1) as c5p,
            tc.tile_pool(name="curB", bufs=1) as cbp,
            tc.tile_pool(name="outL", bufs=3) as olp,
            tc.tile_pool(name="outR", bufs=3) as orp,
        ):
            st = sp.tile([P, U, NS], f16, name="st")
            nc.scalar.dma_start(out=st, in_=xs[:, :, :])

            t68, t9 = {}, {}

            def load68(c):
                if c >= len(CHUNKS):
                    return
                g, off = CHUNKS[c], OFFS[c]
                t68[c] = x68p.tile([P, g, N68], f16, tag="x68", name=f"x68_{c}")
                nc.scalar.dma_start(out=t68[c], in_=x68[:, off : off + g, :])

            def load9(c):
                if c >= len(CHUNKS):
                    return
                g, off = CHUNKS[c], OFFS[c]
                t9[c] = x9p.tile([P, g, N9], f16, tag="x9", name=f"x9_{c}")
                nc.scalar.dma_start(out=t9[c], in_=x9[:, off : off + g, :])

            load68(0)
            load9(0)
            load68(1)
            load9(1)

            # stage A: levels 0..5 for all 64 units at once
            c5 = c5p.tile([P, U, 1 << SPLIT_D], f16, name="c5")
            cur = None
            for d in range(SPLIT_D):
                L = 1 << d
                if d == SPLIT_D - 1:
                    nxt = c5
                else:
                    # stage-A ping-pong borrows the stage-B slots (tags
                    # B0/B1, 4/8 KB); stage A is done before stage B's
                    # first WAR on them
                    nxt = cbp.tile([P, U, 2 * L], f16, tag=f"B{d % 2}")
                left = nxt[:, :, 0:L]
                right = nxt[:, :, L : 2 * L]
                a = st[:, :, L - 1 : 2 * L - 1]
                if d == 0:
                    nc.vector.tensor_copy(out=left, in_=a)
                    nc.vector.tensor_scalar(
                        out=right,
                        in0=a,
                        scalar1=-1.0,
                        scalar2=1.0,
                        op0=mybir.AluOpType.mult,
                        op1=mybir.AluOpType.add,
                    )
                else:
                    nc.vector.tensor_mul(out=left, in0=cur, in1=a)
                    nc.vector.tensor_tensor(
                        out=right, in0=cur, in1=left, op=mybir.AluOpType.subtract
                    )
                cur = nxt

            # stage B: levels 6..9 per chunk of g units
            for c, g in enumerate(CHUNKS):
                off = OFFS[c]
                load68(c + 2)
                load9(c + 2)
                xt = t68.pop(c)
                x9t = t9.pop(c)
                outL = olp.tile([P, g, H], f16, tag="yl")
                outR = orp.tile([P, g, H], f16, tag="yr")
                cur = c5[:, off : off + g, :]
                for d in range(SPLIT_D, TREE_DEPTH):
                    L = 1 << d
                    if d == TREE_DEPTH - 1:
                        a = x9t[:, :, :]
                        # left half of the leaves is final after this mul:
                        # drain it while the subtract computes the right half
                        nc.vector.tensor_mul(out=outL, in0=cur, in1=a)
                        base = off * N_LEAVES
                        nc.sync.dma_start(
                            out=y[:, base : base + g * H].rearrange(
                                "p (u m) -> p u m", u=g, m=H
                            ),
                            in_=outL,
                        )
                        # in-place: per-element read precedes write on DVE
                        nc.vector.tensor_tensor(
                            out=outR, in0=cur, in1=outL, op=mybir.AluOpType.subtract
                        )
                        nc.sync.dma_start(
                            out=y[:, base + g * H : base + 2 * g * H].rearrange(
                                "p (u m) -> p u m", u=g, m=H
                            ),
                            in_=outR,
                        )
                    elif d == TREE_DEPTH - 2:
                        a = xt[:, :, L - 64 : 2 * L - 64]
                        # level-8 output goes straight into the out-right tile
                        left = outR[:, :, 0:L]
                        right = outR[:, :, L : 2 * L]
                        nc.vector.tensor_mul(out=left, in0=cur, in1=a)
                        nc.vector.tensor_tensor(
                            out=right, in0=cur, in1=left, op=mybir.AluOpType.subtract
                        )
                        cur = outR
                    else:
                        a = xt[:, :, L - 64 : 2 * L - 64]
                        nxt = cbp.tile([P, g, 2 * L], f16, tag=f"B{d % 2}")
                        left = nxt[:, :, 0:L]
                        right = nxt[:, :, L : 2 * L]
                        nc.vector.tensor_mul(out=left, in0=cur, in1=a)
                        nc.vector.tensor_tensor(
                            out=right, in0=cur, in1=left, op=mybir.AluOpType.subtract
                        )
                        cur = nxt

    nc.compile()
    return nc


def _run(x: np.ndarray, **spmd_kwargs):
    """Shard x, run the Bass kernel on all 8 cores, return (y, BassKernelResults)."""
    x = np.asarray(x, dtype=np.float32)
    B = x.shape[0]
    assert B % N_CORES == 0 and x.shape[1] == N_NODES
    rpc = B // N_CORES
    assert rpc == P * U

    xh = np.ascontiguousarray(x[:, COL_PERM].astype(np.float16))

    nc = build_nc()
    in_maps = []
    for i in range(N_CORES):
        x3 = xh[i * rpc : (i + 1) * rpc].reshape(P, U, N_NODES)
        in_maps.append(
            {
                "xs": np.ascontiguousarray(x3[:, :, :NS]),
                "x68": np.ascontiguousarray(x3[:, :, NS : NS + N68]),
                "x9": np.ascontiguousarray(x3[:, :, NS + N68 :]),
            }
        )
    res = run_bass_kernel_spmd(nc, in_maps, list(range(N_CORES)), **spmd_kwargs)

    outs = []
    for r in res.results:
        yd = r["y"].reshape(P, U * N_LEAVES)
        yb = np.empty((P, U, N_LEAVES), dtype=np.float16)
        for c, g in enumerate(CHUNKS):
            u0 = OFFS[c]
            seg = yd[:, u0 * N_LEAVES : (u0 + g) * N_LEAVES].reshape(P, 2, g, H)
            yb[:, u0 : u0 + g, 0:H] = seg[:, 0]
            yb[:, u0 : u0 + g, H:] = seg[:, 1]
        outs.append(yb.reshape(rpc, N_LEAVES))
    out = np.concatenate(outs, axis=0)
    out = out[:, OUT_PERM].astype(np.float32)
    return out, res


def kernel(x: np.ndarray) -> np.ndarray:
    return _run(x)[0]
